# revision 1
# baseline (speedup 1.0000x reference)
"""Trainium2 Bass kernel: per-point 3x3 Gaussian covariance from quaternion + log_scale.

cov = R diag(exp(log_scale)) R^T  with R built from the normalized quaternion.

Layout (per core): points sharded [128 partitions, R rows]; tiles of F points
per partition; all DMAs per-partition contiguous.  Normalization folded via
inv2 = 2/|q|^2 (computed fp32 as exp(-ln(n2/2))); the multiply-heavy chain
(products -> R -> M -> Gram) runs in bf16 with contiguous step-1 operands so
VectorE hits its 2x perf mode; ScalarE does the strided deinterleave/cast,
squares, exp/ln, and output interleave.
"""

import os
import numpy as np

import concourse.bass as bass
import concourse.bacc as bacc
import concourse.mybir as mybir
from concourse.tile import TileContext
from concourse.bass_utils import run_bass_kernel_spmd

AF = mybir.ActivationFunctionType
FP32 = mybir.dt.float32
BF16 = mybir.dt.bfloat16

N_CORES = 8
N_FULL = 4_000_000
P = 128
R = 3908                      # rows per partition per core; 128*3908*8 = 4_001_792 >= N
NPC = P * R                   # points per core (padded)
F = int(os.environ.get("KERNEL_F", "448"))  # points per partition per tile

SQRT_HALF = 0.7071067811865476

_built = {}


def _build():
    key = F
    if key in _built:
        return _built[key]

    nc = bacc.Bacc("TRN2", target_bir_lowering=False, debug=False, num_devices=N_CORES)
    q = nc.dram_tensor("q", [NPC, 4], FP32, kind="ExternalInput")
    ls = nc.dram_tensor("ls", [NPC, 3], FP32, kind="ExternalInput")
    cov = nc.dram_tensor("cov", [NPC, 3, 3], FP32, kind="ExternalOutput")

    qv = q.ap().rearrange("(p r) c -> p (r c)", p=P)       # [128, 4R]
    lsv = ls.ap().rearrange("(p r) c -> p (r c)", p=P)     # [128, 3R]
    ov = cov.ap().rearrange("(p r) i k -> p (r i k)", p=P)  # [128, 9R]

    with TileContext(nc) as tc:
        with (
            tc.tile_pool(name="io", bufs=2) as io,
            tc.tile_pool(name="otp", bufs=2) as ot_pool,
            tc.tile_pool(name="big", bufs=2) as big,
            tc.tile_pool(name="wk", bufs=2) as wk,
        ):
            t0 = 0
            while t0 < R:
                f = min(F, R - t0)
                _tile_body(nc, io, ot_pool, big, wk, qv, lsv, ov, t0, f)
                t0 += f

    nc.compile()
    _built[key] = nc
    return nc


def _tile_body(nc, io, ot_pool, big, wk, qv, lsv, ov, t0, f):
    cnt = [0]

    def w(dt=BF16, tag=None):
        cnt[0] += 1
        tag = tag or f"w{cnt[0]}"
        return wk.tile([P, f], dt, tag=tag, name=f"{tag}_t{t0}_{cnt[0]}")

    qt = io.tile([P, 4 * f], FP32, tag="qt", name=f"qt{t0}")
    lst = io.tile([P, 3 * f], FP32, tag="lst", name=f"lst{t0}")
    nc.sync.dma_start(out=qt, in_=qv[:, 4 * t0:4 * (t0 + f)])
    nc.sync.dma_start(out=lst, in_=lsv[:, 3 * t0:3 * (t0 + f)])

    qc = qt.rearrange("p (f c) -> p f c", c=4)
    lsc = lst.rearrange("p (f c) -> p f c", c=3)

    # ---- fp32 path: n2/2 and inv2 = 2/|q|^2 = exp(-ln(n2/2)) -------------
    sq4 = big.tile([P, 4 * f], FP32, tag="sq4", name=f"sq4_{t0}")
    nc.scalar.activation(sq4, qt, AF.Square, scale=SQRT_HALF)  # x^2/2
    sqc = sq4.rearrange("p (f c) -> p f c", c=4)
    u = w(FP32, tag="fu"); v = w(FP32, tag="fv"); n2h = w(FP32, tag="fn2h")
    lnv = w(FP32, tag="fu"); inv2 = w(FP32, tag="fv")
    nc.vector.tensor_add(u, sqc[:, :, 0], sqc[:, :, 1])
    nc.vector.tensor_add(v, sqc[:, :, 2], sqc[:, :, 3])
    nc.vector.tensor_add(n2h, u, v)
    nc.scalar.activation(lnv, n2h, AF.Ln)
    nc.scalar.activation(inv2, lnv, AF.Exp, scale=-1.0)

    # ---- deinterleave + cast to bf16 (ScalarE, strided reads) ------------
    a_ = w(); b_ = w(); c_ = w(); d_ = w(); ivb = w()
    nc.scalar.copy(out=a_, in_=qc[:, :, 0])
    nc.scalar.copy(out=b_, in_=qc[:, :, 1])
    nc.scalar.copy(out=c_, in_=qc[:, :, 2])
    nc.scalar.copy(out=d_, in_=qc[:, :, 3])
    nc.scalar.copy(out=ivb, in_=inv2)

    # ---- bf16 chain: A..D, products (VectorE 2x mode) --------------------
    A = w(); B = w(); C = w(); D = w()
    nc.vector.tensor_mul(A, ivb, a_)
    nc.vector.tensor_mul(B, ivb, b_)
    nc.vector.tensor_mul(C, ivb, c_)
    nc.vector.tensor_mul(D, ivb, d_)

    Ab = w(); Ac = w(); Ad = w()
    Bb = w(); Bc = w(); Bd = w()
    Cc = w(); Cd = w(); Dd = w()
    nc.vector.tensor_mul(Ab, A, b_)
    nc.vector.tensor_mul(Ac, A, c_)
    nc.vector.tensor_mul(Ad, A, d_)
    nc.vector.tensor_mul(Bb, B, b_)
    nc.vector.tensor_mul(Bc, B, c_)
    nc.vector.tensor_mul(Bd, B, d_)
    nc.vector.tensor_mul(Cc, C, c_)
    nc.vector.tensor_mul(Cd, C, d_)
    nc.vector.tensor_mul(Dd, D, d_)

    # ---- rotation matrix entries (bf16) ----------------------------------
    t_0 = w(); t_1 = w(); t_2 = w()
    nc.vector.tensor_add(t_0, Cc, Dd)
    nc.vector.tensor_add(t_1, Bb, Dd)
    nc.vector.tensor_add(t_2, Bb, Cc)
    r00 = w(FP32, tag="fr00"); r11 = w(FP32, tag="fr11"); r22 = w(FP32, tag="fr22")
    nc.scalar.activation(r00, t_0, AF.Identity, bias=1.0, scale=-1.0)
    nc.scalar.activation(r11, t_1, AF.Identity, bias=1.0, scale=-1.0)
    nc.scalar.activation(r22, t_2, AF.Identity, bias=1.0, scale=-1.0)
    r01 = w(); r10 = w(); r02 = w(); r20 = w(); r12 = w(); r21 = w()
    nc.vector.tensor_sub(r01, Bc, Ad)
    nc.vector.tensor_add(r10, Bc, Ad)
    nc.vector.tensor_add(r02, Bd, Ac)
    nc.vector.tensor_sub(r20, Bd, Ac)
    nc.vector.tensor_sub(r12, Cd, Ab)
    nc.vector.tensor_add(r21, Cd, Ab)

    # ---- sqrt(scale) per column (ScalarE, bf16 contiguous out) -----------
    sh = [w(FP32, tag="fsh0"), w(FP32, tag="fsh1"), w(FP32, tag="fsh2")]
    for j in range(3):
        nc.scalar.activation(sh[j], lsc[:, :, j], AF.Exp, scale=0.5)

    Rm = [[r00, r01, r02], [r10, r11, r12], [r20, r21, r22]]
    M = [[None] * 3 for _ in range(3)]
    for i in range(3):
        for j in range(3):
            M[i][j] = w(FP32 if i == j else BF16, tag=f"pm{i}{j}")
            nc.vector.tensor_mul(M[i][j], Rm[i][j], sh[j])

    # ---- cov = M M^T; diag entries write straight into the out tile ------
    ot = ot_pool.tile([P, 9 * f], FP32, tag="ot", name=f"ot_{t0}")
    otv = ot.rearrange("p (f e) -> p f e", e=9)
    offd = {}
    for (i, k) in [(0, 0), (0, 1), (0, 2), (1, 1), (1, 2), (2, 2)]:
        fd = i == k
        g = w(FP32 if fd else BF16, tag="ggf" if fd else "gg")
        g2 = w(FP32 if fd else BF16, tag="gg2f" if fd else "gg2")
        h = w(tag="gh"); h2 = w(tag="gh2")
        nc.vector.tensor_mul(g, M[i][0], M[k][0])
        nc.vector.tensor_mul(h, M[i][1], M[k][1])
        nc.vector.tensor_add(g2, g, h)
        nc.vector.tensor_mul(h2, M[i][2], M[k][2])
        if i == k:
            nc.vector.tensor_add(otv[:, :, 3 * i + k], g2, h2)  # fp32 strided out
        else:
            cik = w(tag=f"cov{i}{k}")
            nc.vector.tensor_add(cik, g2, h2)
            offd[(i, k)] = cik

    # off-diagonals + symmetric duplicates via ScalarE copies (cast to fp32)
    for (i, k), cik in offd.items():
        nc.scalar.copy(out=otv[:, :, 3 * i + k], in_=cik)
        nc.scalar.copy(out=otv[:, :, 3 * k + i], in_=cik)

    nc.sync.dma_start(out=ov[:, 9 * t0:9 * (t0 + f)], in_=ot)


def _pad_and_shard(quaternion, log_scale):
    n = quaternion.shape[0]
    pad = N_CORES * NPC - n
    if pad:
        qpad = np.tile(np.array([1, 0, 0, 0], np.float32), (pad, 1))
        lpad = np.zeros((pad, 3), np.float32)
        quaternion = np.concatenate([quaternion, qpad], axis=0)
        log_scale = np.concatenate([log_scale, lpad], axis=0)
    in_maps = []
    for i in range(N_CORES):
        sl = slice(i * NPC, (i + 1) * NPC)
        in_maps.append({
            "q": np.ascontiguousarray(quaternion[sl]),
            "ls": np.ascontiguousarray(log_scale[sl]),
        })
    return in_maps


def kernel_with_stats(quaternion, log_scale, trace=False):
    quaternion = np.asarray(quaternion, dtype=np.float32)
    log_scale = np.asarray(log_scale, dtype=np.float32)
    n = quaternion.shape[0]
    nc = _build()
    in_maps = _pad_and_shard(quaternion, log_scale)
    res = run_bass_kernel_spmd(nc, in_maps, core_ids=list(range(N_CORES)), trace=trace)
    out = np.concatenate([r["cov"] for r in res.results], axis=0)[:n]
    return out, res


def kernel(quaternion, log_scale):
    out, _ = kernel_with_stats(quaternion, log_scale, trace=False)
    return out



# revision 2
# speedup vs baseline: 2.1364x; 2.1364x over previous
"""Trainium2 Bass kernel: per-point 3x3 Gaussian covariance from quaternion + log_scale.

cov = R diag(exp(log_scale)) R^T with R built from the normalized quaternion.

Strategy (v2, planar fp16):
  * Host reshapes inputs to struct-of-arrays fp16 planes per core:
    q [128, 4, R], ls [128, 3, R]; device writes the 6 unique entries of the
    symmetric cov as fp16 planes [128, 6, R]; host mirrors/casts to [N,3,3] f32.
  * Math: with half-square sums x0=(a^2+b^2-c^2-d^2)/2 etc. and unnormalized
    rotation half-columns x=(x0, bc+ad, bd-ac), y=(bc-ad, y1, cd+ab):
        cov = s2*I + alpha * x x^T + beta * y y^T
    where alpha=(s0-s2)*4/n^4, beta=(s1-s2)*4/n^4 arrive for free via
    e_j = exp(ls_j - 2*ln(n^2/2)) = s_j*4/n^4  (only TWO outer products thanks
    to sum_j r_j r_j^T = I).
  * All DVE ops are unit-stride fp16 [128, F] tiles -> 2x perf mode.
    ScalarE does squares/ln/exp; GpSimd takes a few off-critical adds.
"""

import os
import numpy as np

import concourse.bass as bass
import concourse.bacc as bacc
import concourse.mybir as mybir
from concourse.tile import TileContext
from concourse.bass_utils import run_bass_kernel_spmd

AF = mybir.ActivationFunctionType
OP = mybir.AluOpType
FP16 = mybir.dt.float16
FP32 = mybir.dt.float32

N_CORES = 8
N_FULL = 4_000_000
P = 128
R = 3912                      # rows per partition per core; 128*3912*8 >= 4M
NPC = P * R                   # points per core (padded)
F = int(os.environ.get("KERNEL_F", "978"))   # points per partition per tile
USE_STT = os.environ.get("KERNEL_STT", "1") == "1"
GPS = os.environ.get("KERNEL_GPS", "1") == "1"   # offload some adds to gpsimd

SQRT_HALF = 0.7071067811865476

# output plane order: (i,k) pairs of the symmetric cov
PAIRS = [(0, 0), (0, 1), (0, 2), (1, 1), (1, 2), (2, 2)]

_built = {}


def _build():
    key = (F, USE_STT, GPS)
    if key in _built:
        return _built[key]

    nc = bacc.Bacc("TRN2", target_bir_lowering=False, debug=False, num_devices=N_CORES)
    q = nc.dram_tensor("q", [P, 4, R], FP16, kind="ExternalInput")
    ls = nc.dram_tensor("ls", [P, 3, R], FP16, kind="ExternalInput")
    cov = nc.dram_tensor("cov", [P, 6, R], FP16, kind="ExternalOutput")

    qv = q.ap()
    lsv = ls.ap()
    ov = cov.ap()

    with TileContext(nc) as tc:
        with (
            tc.tile_pool(name="io", bufs=2) as io,
            tc.tile_pool(name="otp", bufs=2) as otp,
            tc.tile_pool(name="wk2", bufs=2) as wk2,
            tc.tile_pool(name="wk1", bufs=1) as wk1,
        ):
            t0 = 0
            while t0 < R:
                f = min(F, R - t0)
                _tile_body(nc, io, otp, wk2, wk1, qv, lsv, ov, t0, f)
                t0 += f

    nc.compile()
    _built[key] = nc
    return nc


def _tile_body(nc, io, otp, wk2, wk1, qv, lsv, ov, t0, f):
    def w2(tag):
        return wk2.tile([P, f], FP16, tag=tag, name=f"{tag}_{t0}")

    def w1(tag):
        return wk1.tile([P, f], FP16, tag=tag, name=f"{tag}_{t0}")

    V = nc.vector
    G = nc.gpsimd if GPS else nc.vector

    qt = io.tile([P, 4 * f], FP16, tag="qt", name=f"qt{t0}")
    lst = io.tile([P, 3 * f], FP16, tag="lst", name=f"lst{t0}")
    nc.sync.dma_start(out=qt.rearrange("p (c f) -> p c f", c=4), in_=qv[:, :, t0:t0 + f])
    nc.sync.dma_start(out=lst.rearrange("p (c f) -> p c f", c=3), in_=lsv[:, :, t0:t0 + f])

    a = qt[:, 0:f]; b = qt[:, f:2 * f]; c = qt[:, 2 * f:3 * f]; d = qt[:, 3 * f:4 * f]
    l0 = lst[:, 0:f]; l1 = lst[:, f:2 * f]; l2 = lst[:, 2 * f:3 * f]

    # --- squares (ScalarE): s* = (comp^2)/2 ------------------------------
    sa = w2("sa"); sb = w2("sb"); sc = w2("sc"); sd = w2("sd")
    nc.scalar.activation(sa, a, AF.Square, scale=SQRT_HALF)
    nc.scalar.activation(sb, b, AF.Square, scale=SQRT_HALF)
    nc.scalar.activation(sc, c, AF.Square, scale=SQRT_HALF)
    nc.scalar.activation(sd, d, AF.Square, scale=SQRT_HALF)

    # --- half-square combos ----------------------------------------------
    u = w1("u"); v = w1("v"); n2h = w2("n2h"); x0 = w1("x0"); y1 = w1("y1")
    V.tensor_add(u, sa, sb)
    V.tensor_add(v, sc, sd)
    V.tensor_add(n2h, u, v)
    V.tensor_sub(x0, u, v)
    if USE_STT:
        z = w2("z")
        G.tensor_add(z, sb, sd)
        # y1 = n2h - 2*z
        V.scalar_tensor_tensor(out=y1, in0=z, scalar=-2.0, in1=n2h,
                               op0=OP.mult, op1=OP.add)
    else:
        u2 = w1("u2"); v2 = w1("v2")
        V.tensor_sub(u2, sa, sb)
        V.tensor_sub(v2, sc, sd)
        V.tensor_add(y1, u2, v2)

    # --- normalization scalars (ScalarE ln/exp) --------------------------
    ln = w2("ln")
    nc.scalar.activation(ln, n2h, AF.Ln)
    t_ = [w2(f"t{j}") for j in range(3)]
    lj = [l0, l1, l2]
    if USE_STT:
        for j in range(3):
            # t_j = ls_j - 2*ln(n2h)
            V.scalar_tensor_tensor(out=t_[j], in0=ln, scalar=-2.0, in1=lj[j],
                                   op0=OP.mult, op1=OP.add)
    else:
        lnm2 = w2("lnm2")
        nc.scalar.mul(lnm2, ln, -2.0)
        for j in range(3):
            V.tensor_add(t_[j], lnm2, lj[j])
    e0 = w2("e0"); e1 = w2("e1"); e2 = w2("e2"); s2t = w2("s2t")
    nc.scalar.activation(e0, t_[0], AF.Exp)
    nc.scalar.activation(e1, t_[1], AF.Exp)
    nc.scalar.activation(e2, t_[2], AF.Exp)
    nc.scalar.activation(s2t, l2, AF.Exp)

    # --- quaternion cross products + rotation half-columns ----------------
    pbc = w1("pbc"); pad = w1("pad"); pbd = w1("pbd")
    pac = w1("pac"); pcd = w1("pcd"); pab = w1("pab")
    V.tensor_mul(pbc, b, c)
    V.tensor_mul(pad, a, d)
    V.tensor_mul(pbd, b, d)
    V.tensor_mul(pac, a, c)
    V.tensor_mul(pcd, c, d)
    V.tensor_mul(pab, a, b)
    x1 = w1("x1"); y0 = w1("y0"); x2 = w1("x2"); y2 = w1("y2")
    V.tensor_add(x1, pbc, pad)
    V.tensor_sub(y0, pbc, pad)
    V.tensor_sub(x2, pbd, pac)
    V.tensor_add(y2, pcd, pab)

    # --- alpha/beta and weighted columns ----------------------------------
    al = w1("al"); be = w1("be")
    V.tensor_sub(al, e0, e2)
    V.tensor_sub(be, e1, e2)
    X = [x0, x1, x2]
    Y = [y0, y1, y2]
    W0 = [w1(f"w0{i}") for i in range(3)]
    W1 = [w1(f"w1{i}") for i in range(3)]
    for i in range(3):
        V.tensor_mul(W0[i], al, X[i])
        V.tensor_mul(W1[i], be, Y[i])

    # --- cov entries, written straight into the out tile ------------------
    ot = otp.tile([P, 6 * f], FP16, tag="ot", name=f"ot{t0}")
    g0 = w1("g0"); h0 = w1("h0"); g1 = w1("g1"); h1 = w1("h1")
    dt0 = w2("dt0"); dt1 = w2("dt1")
    for idx, (i, k) in enumerate(PAIRS):
        g, h = (g0, h0) if idx % 2 == 0 else (g1, h1)
        tgt = ot[:, idx * f:(idx + 1) * f]
        V.tensor_mul(g, W0[i], X[k])
        V.tensor_mul(h, W1[i], Y[k])
        if i == k:
            dt = dt0 if i % 2 == 0 else dt1
            V.tensor_add(dt, g, h)
            G.tensor_add(tgt, dt, s2t)
        else:
            V.tensor_add(tgt, g, h)

    nc.sync.dma_start(out=ov[:, :, t0:t0 + f], in_=ot.rearrange("p (c f) -> p c f", c=6))


def _pack_inputs(quaternion, log_scale):
    n = quaternion.shape[0]
    total = N_CORES * NPC
    qp = np.empty((total, 4), np.float16)
    lp = np.empty((total, 3), np.float16)
    qp[:n] = quaternion[:n]
    lp[:n] = log_scale[:n]
    if total > n:
        qp[n:] = np.array([1, 0, 0, 0], np.float16)
        lp[n:] = 0
    in_maps = []
    for i in range(N_CORES):
        sl = slice(i * NPC, (i + 1) * NPC)
        qc = np.ascontiguousarray(
            qp[sl].reshape(P, R, 4).transpose(0, 2, 1))
        lc = np.ascontiguousarray(
            lp[sl].reshape(P, R, 3).transpose(0, 2, 1))
        in_maps.append({"q": qc, "ls": lc})
    return in_maps


def _unpack_output(results, n):
    # device planes: [P, 6, R] fp16, order PAIRS
    planes = np.concatenate(
        [r["cov"].transpose(0, 2, 1).reshape(NPC, 6) for r in results], axis=0
    )[:n].astype(np.float32)
    out = np.empty((n, 3, 3), np.float32)
    out[:, 0, 0] = planes[:, 0]
    out[:, 0, 1] = planes[:, 1]; out[:, 1, 0] = planes[:, 1]
    out[:, 0, 2] = planes[:, 2]; out[:, 2, 0] = planes[:, 2]
    out[:, 1, 1] = planes[:, 3]
    out[:, 1, 2] = planes[:, 4]; out[:, 2, 1] = planes[:, 4]
    out[:, 2, 2] = planes[:, 5]
    return out


def kernel_with_stats(quaternion, log_scale, trace=False):
    quaternion = np.asarray(quaternion, dtype=np.float32)
    log_scale = np.asarray(log_scale, dtype=np.float32)
    n = quaternion.shape[0]
    nc = _build()
    in_maps = _pack_inputs(quaternion, log_scale)
    res = run_bass_kernel_spmd(nc, in_maps, core_ids=list(range(N_CORES)), trace=trace)
    out = _unpack_output(res.results, n)
    return out, res


def kernel(quaternion, log_scale):
    out, _ = kernel_with_stats(quaternion, log_scale, trace=False)
    return out


# revision 3
# speedup vs baseline: 2.4366x; 1.1405x over previous
"""Trainium2 Bass kernel: per-point 3x3 Gaussian covariance from quaternion + log_scale.

cov = R diag(exp(log_scale)) R^T with R built from the normalized quaternion.

Strategy (v3, planar fp16):
  * Host reshapes inputs to struct-of-arrays fp16 planes per core:
    q [128, 4, R], ls [128, 3, R]; device writes the 6 unique entries of the
    symmetric cov as fp16 planes [128, 6, R]; host mirrors/casts to [N,3,3] f32.
  * Math: with half-square sums x0=(a^2+b^2-c^2-d^2)/2 etc. and unnormalized
    rotation half-columns x=(x0, bc+ad, bd-ac), y=(bc-ad, y1, cd+ab):
        cov = s2*I + alpha * x x^T + beta * y y^T
    where alpha=(s0-s2)*4/n^4, beta=(s1-s2)*4/n^4; the 4/n^4 comes via
    inv4 = exp(-2*ln(n^2/2)). Only TWO outer products thanks to
    sum_j r_j r_j^T = I.
  * All DVE ops are unit-stride fp16 [128, F] tiles -> 2x perf mode.
    ScalarE does squares/ln/exp; GpSimd owns the (2,2) gram entry and the
    diagonal +s2 adds as an independent chain.
"""

import os
import numpy as np

import concourse.bass as bass
import concourse.bacc as bacc
import concourse.mybir as mybir
from concourse.tile import TileContext
from concourse.bass_utils import run_bass_kernel_spmd

AF = mybir.ActivationFunctionType
OP = mybir.AluOpType
FP16 = mybir.dt.float16
FP32 = mybir.dt.float32

N_CORES = 8
N_FULL = 4_000_000
P = 128
R = 3912                      # rows per partition per core; 128*3912*8 >= 4M
NPC = P * R                   # points per core (padded)
F = int(os.environ.get("KERNEL_F", "978"))   # points per partition per tile
GPS = int(os.environ.get("KERNEL_GPS", "2"))  # gpsimd offload level 0/1/2

SQRT_HALF = 0.7071067811865476

# output plane order: (i,k) pairs of the symmetric cov
PAIRS = [(0, 0), (0, 1), (0, 2), (1, 1), (1, 2), (2, 2)]

_built = {}


def _build():
    key = (F, GPS)
    if key in _built:
        return _built[key]

    nc = bacc.Bacc("TRN2", target_bir_lowering=False, debug=False, num_devices=N_CORES)
    q = nc.dram_tensor("q", [P, 4, R], FP16, kind="ExternalInput")
    ls = nc.dram_tensor("ls", [P, 3, R], FP16, kind="ExternalInput")
    cov = nc.dram_tensor("cov", [P, 6, R], FP16, kind="ExternalOutput")

    qv = q.ap()
    lsv = ls.ap()
    ov = cov.ap()

    with TileContext(nc) as tc:
        with (
            tc.tile_pool(name="io", bufs=2) as io,
            tc.tile_pool(name="otp", bufs=2) as otp,
            tc.tile_pool(name="wk2", bufs=2) as wk2,
            tc.tile_pool(name="wk1", bufs=1) as wk1,
        ):
            t0 = 0
            while t0 < R:
                f = min(F, R - t0)
                _tile_body(nc, io, otp, wk2, wk1, qv, lsv, ov, t0, f)
                t0 += f

    nc.compile()
    _built[key] = nc
    return nc


def _tile_body(nc, io, otp, wk2, wk1, qv, lsv, ov, t0, f):
    def w2(tag):
        return wk2.tile([P, f], FP16, tag=tag, name=f"{tag}_{t0}")

    def w1(tag):
        return wk1.tile([P, f], FP16, tag=tag, name=f"{tag}_{t0}")

    V = nc.vector
    G = nc.gpsimd if GPS else nc.vector

    qt = io.tile([P, 4 * f], FP16, tag="qt", name=f"qt{t0}")
    lst = io.tile([P, 3 * f], FP16, tag="lst", name=f"lst{t0}")
    nc.sync.dma_start(out=qt.rearrange("p (c f) -> p c f", c=4), in_=qv[:, :, t0:t0 + f])
    nc.sync.dma_start(out=lst.rearrange("p (c f) -> p c f", c=3), in_=lsv[:, :, t0:t0 + f])

    a = qt[:, 0:f]; b = qt[:, f:2 * f]; c = qt[:, 2 * f:3 * f]; d = qt[:, 3 * f:4 * f]
    l0 = lst[:, 0:f]; l1 = lst[:, f:2 * f]; l2 = lst[:, 2 * f:3 * f]

    # --- scale exps (ScalarE, only needs lst) ----------------------------
    s0t = w2("s0t"); s1t = w2("s1t"); s2t = w2("s2t")
    nc.scalar.activation(s0t, l0, AF.Exp)
    nc.scalar.activation(s1t, l1, AF.Exp)
    nc.scalar.activation(s2t, l2, AF.Exp)

    # --- quaternion cross products (DVE, only needs qt) -------------------
    pbc = w1("pbc"); pad = w1("pad"); pbd = w1("pbd")
    pac = w1("pac"); pcd = w1("pcd"); pab = w1("pab")
    V.tensor_mul(pbc, b, c)
    V.tensor_mul(pad, a, d)
    V.tensor_mul(pbd, b, d)
    V.tensor_mul(pac, a, c)
    V.tensor_mul(pcd, c, d)
    V.tensor_mul(pab, a, b)
    x1 = w1("x1"); y0 = w1("y0"); x2 = w1("x2"); y2 = w1("y2")
    V.tensor_add(x1, pbc, pad)
    V.tensor_sub(y0, pbc, pad)
    V.tensor_sub(x2, pbd, pac)
    V.tensor_add(y2, pcd, pab)

    # --- squares (ScalarE): s* = (comp^2)/2 ------------------------------
    sa = w2("sa"); sb = w2("sb"); sc = w2("sc"); sd = w2("sd")
    nc.scalar.activation(sa, a, AF.Square, scale=SQRT_HALF)
    nc.scalar.activation(sb, b, AF.Square, scale=SQRT_HALF)
    nc.scalar.activation(sc, c, AF.Square, scale=SQRT_HALF)
    nc.scalar.activation(sd, d, AF.Square, scale=SQRT_HALF)

    # --- half-square combos (DVE) -----------------------------------------
    u = w1("u"); v = w1("v"); n2h = w2("n2h"); x0 = w1("x0")
    u2 = w1("u2"); v2 = w1("v2"); y1 = w1("y1")
    V.tensor_add(u, sa, sb)
    V.tensor_add(v, sc, sd)
    V.tensor_add(n2h, u, v)
    V.tensor_sub(x0, u, v)
    V.tensor_sub(u2, sa, sb)
    V.tensor_sub(v2, sc, sd)
    V.tensor_add(y1, u2, v2)

    # --- normalization: inv4 = 4/n^4 = exp(-2*ln(n^2/2)) (ScalarE chain) --
    ln = w2("ln"); inv4 = w2("inv4")
    nc.scalar.activation(ln, n2h, AF.Ln)
    nc.scalar.activation(inv4, ln, AF.Exp, scale=-2.0)

    # --- alpha/beta and weighted columns (DVE) ----------------------------
    d0 = w1("d0"); d1 = w1("d1"); al = w1("al"); be = w1("be")
    V.tensor_sub(d0, s0t, s2t)
    V.tensor_sub(d1, s1t, s2t)
    V.tensor_mul(al, d0, inv4)
    V.tensor_mul(be, d1, inv4)
    X = [x0, x1, x2]
    Y = [y0, y1, y2]
    W0 = [w1(f"w0{i}") for i in range(3)]
    W1 = [w1(f"w1{i}") for i in range(3)]
    # order: i=2 first so gpsimd's (2,2) chain can start early
    for i in (2, 0, 1):
        V.tensor_mul(W0[i], al, X[i])
        V.tensor_mul(W1[i], be, Y[i])

    # --- cov entries, written straight into the out tile ------------------
    # plane idx per PAIRS; (2,2) fully on gpsimd; diag +s2 on gpsimd
    ot = otp.tile([P, 6 * f], FP16, tag="ot", name=f"ot{t0}")

    def plane(idx):
        return ot[:, idx * f:(idx + 1) * f]

    if GPS >= 2:
        g22 = w2("g22"); h22 = w2("h22"); q22 = w2("q22")
        G.tensor_mul(g22, W0[2], X[2])
        G.tensor_mul(h22, W1[2], Y[2])
        G.tensor_add(q22, g22, h22)
        G.tensor_add(plane(5), q22, s2t)
    else:
        g22 = w1("g22"); h22 = w1("h22"); q22 = w2("q22")
        V.tensor_mul(g22, W0[2], X[2])
        V.tensor_mul(h22, W1[2], Y[2])
        V.tensor_add(q22, g22, h22)
        G.tensor_add(plane(5), q22, s2t)

    # diag (0,0) and (1,1): DVE muls+add, gpsimd final +s2
    g0 = w1("g0"); h0 = w1("h0"); g1 = w1("g1"); h1 = w1("h1")
    dt0 = w2("dt0"); dt1 = w2("dt1")
    V.tensor_mul(g0, W0[0], X[0])
    V.tensor_mul(h0, W1[0], Y[0])
    V.tensor_add(dt0, g0, h0)
    G.tensor_add(plane(0), dt0, s2t)
    V.tensor_mul(g1, W0[1], X[1])
    V.tensor_mul(h1, W1[1], Y[1])
    V.tensor_add(dt1, g1, h1)
    G.tensor_add(plane(3), dt1, s2t)

    # off-diagonals fully on DVE
    for idx, (i, k) in [(1, (0, 1)), (2, (0, 2)), (4, (1, 2))]:
        g, h = (g0, h0) if idx % 2 == 0 else (g1, h1)
        V.tensor_mul(g, W0[i], X[k])
        V.tensor_mul(h, W1[i], Y[k])
        V.tensor_add(plane(idx), g, h)

    nc.sync.dma_start(out=ov[:, :, t0:t0 + f], in_=ot.rearrange("p (c f) -> p c f", c=6))


def _pack_inputs(quaternion, log_scale):
    n = quaternion.shape[0]
    total = N_CORES * NPC
    qp = np.empty((total, 4), np.float16)
    lp = np.empty((total, 3), np.float16)
    qp[:n] = quaternion[:n]
    lp[:n] = log_scale[:n]
    if total > n:
        qp[n:] = np.array([1, 0, 0, 0], np.float16)
        lp[n:] = 0
    in_maps = []
    for i in range(N_CORES):
        sl = slice(i * NPC, (i + 1) * NPC)
        qc = np.ascontiguousarray(
            qp[sl].reshape(P, R, 4).transpose(0, 2, 1))
        lc = np.ascontiguousarray(
            lp[sl].reshape(P, R, 3).transpose(0, 2, 1))
        in_maps.append({"q": qc, "ls": lc})
    return in_maps


def _unpack_output(results, n):
    # device planes: [P, 6, R] fp16, order PAIRS
    planes = np.concatenate(
        [r["cov"].transpose(0, 2, 1).reshape(NPC, 6) for r in results], axis=0
    )[:n].astype(np.float32)
    out = np.empty((n, 3, 3), np.float32)
    out[:, 0, 0] = planes[:, 0]
    out[:, 0, 1] = planes[:, 1]; out[:, 1, 0] = planes[:, 1]
    out[:, 0, 2] = planes[:, 2]; out[:, 2, 0] = planes[:, 2]
    out[:, 1, 1] = planes[:, 3]
    out[:, 1, 2] = planes[:, 4]; out[:, 2, 1] = planes[:, 4]
    out[:, 2, 2] = planes[:, 5]
    return out


def kernel_with_stats(quaternion, log_scale, trace=False):
    quaternion = np.asarray(quaternion, dtype=np.float32)
    log_scale = np.asarray(log_scale, dtype=np.float32)
    n = quaternion.shape[0]
    nc = _build()
    in_maps = _pack_inputs(quaternion, log_scale)
    res = run_bass_kernel_spmd(nc, in_maps, core_ids=list(range(N_CORES)), trace=trace)
    out = _unpack_output(res.results, n)
    return out, res


def kernel(quaternion, log_scale):
    out, _ = kernel_with_stats(quaternion, log_scale, trace=False)
    return out
